# revision 16
# baseline (speedup 1.0000x reference)
"""AttentionPooling1D Trainium2 kernel.

Reference computation (per batch element b):
    scores[s] = x[b, s, :] @ w[0]                  # [S]
    scores    = where(mask[b] != 0, scores, -inf)
    probs     = softmax(scores)                    # [S]
    out[b, :] = probs @ x[b, :, :]                 # [D]

Strategy (memory-bound, one pass over x):
  - Shard batch B=64 across 8 cores (8 per core), no communication.
  - Per core, stream x in [128 s-rows, D] chunks (2 chunks = 1 MiB per DMA).
  - DVE tensor_tensor_reduce computes scores = sum_d(x * w_rep) + mask_bias
    in ONE pass (the elementwise product stream is discarded via a stride-0
    output; the per-partition accumulator gives the row dot products).
  - ScalarE exp. Masked rows get bias -30000 -> exp underflows to exactly 0,
    which makes the softmax numerator/denominator drop them, so no -inf or
    max-subtraction is needed (scores ~ N(0,1), exp cannot overflow).
  - TensorE accumulates numerator acc[1, D] += e^T @ x_chunk and the
    denominator l += e^T @ ones in PSUM across all 32 chunks of a batch
    (float32r: fp32 in/out, FP22 multiply, fp32 accumulate, 1 cycle/row).
  - Final: out[b] = acc * (1/l) via ScalarE, DMA to DRAM.

Host-side prep (negligible bytes): broadcast w to [128, D]; turn the int32
mask into the additive f32 bias laid out as [128, batch*chunk] columns.
"""

import numpy as np

B, S, D = 64, 4096, 1024
N_CORES = 8
B_PC = B // N_CORES      # batches per core
P = 128                  # SBUF partitions
NEG_BIAS = -30000.0      # exp(x + NEG_BIAS) == 0.0 in fp32 for any plausible x


def build_bass(b_pc=B_PC, s=S, d=D, super_=4, x_bufs=4, use_bf16=False):
    """Build the single-core Bass program. Parameterized so tests can build
    a small config for CoreSim."""
    import concourse.bacc as bacc
    import concourse.tile as tile
    from concourse import mybir

    cpb = s // P             # chunks per batch
    scpb = cpb // super_     # superchunks per batch
    assert scpb * super_ == cpb and cpb * P == s
    assert d % 1024 == 0 or d <= 1024

    f32 = mybir.dt.float32
    f32r = mybir.dt.float32r
    bf16 = mybir.dt.bfloat16
    # xd: dtype of the streamed x / w / e operands; mm views feed the PE
    xd = bf16 if use_bf16 else f32

    nc = bacc.Bacc(trn_type="TRN2", target_bir_lowering=False, debug=False)
    x_d = nc.declare_dram_parameter("x", [b_pc, s, d], f32, isOutput=False)
    w_d = nc.declare_dram_parameter("w_rep", [P, d], xd, isOutput=False)
    bias_d = nc.declare_dram_parameter("bias", [P, b_pc * cpb], f32, isOutput=False)
    ones_d = nc.declare_dram_parameter("ones", [P, 2], xd, isOutput=False)
    out_d = nc.declare_dram_parameter("out", [b_pc, d], f32, isOutput=True)

    def mm(ap):
        # PE-view of an operand: fp32 operands must be fed as float32r
        # (FP22-truncate-on-read) to stream at 1 cycle/row; bf16 is native.
        return ap if use_bf16 else ap.bitcast(f32r)

    n_half = d // 2          # 512 for the real problem (PSUM fp32 matmul max)
    assert n_half <= 512

    with tile.TileContext(nc) as tc:
        with (
            tc.tile_pool(name="xpool", bufs=x_bufs) as xpool,
            tc.tile_pool(name="ypool", bufs=3) as ypool,
            tc.tile_pool(name="consts", bufs=1) as consts,
            tc.tile_pool(name="small", bufs=8) as small,
            tc.tile_pool(name="outp", bufs=2) as outp,
            tc.tile_pool(name="psum", bufs=2, space="PSUM") as psum_pool,
        ):
            w_sb = consts.tile([P, d], xd)
            nc.sync.dma_start(out=w_sb, in_=w_d[:])
            bias_sb = consts.tile([P, b_pc * cpb], f32)
            nc.sync.dma_start(out=bias_sb, in_=bias_d[:])
            ones_sb = consts.tile([P, 2], xd)
            nc.sync.dma_start(out=mm(ones_sb), in_=mm(ones_d[:]))

            for b in range(b_pc):
                acc0 = psum_pool.tile([1, n_half], f32, tag="acc0")
                acc1 = psum_pool.tile([1, n_half], f32, tag="acc1")
                lps = psum_pool.tile([1, 2], f32, tag="l")
                for sc in range(scpb):
                    xt = xpool.tile([P, super_, d], xd, tag="xt")
                    src = x_d[b, sc * super_ * P : (sc + 1) * super_ * P, :].rearrange(
                        "(j p) d -> p j d", p=P
                    )
                    if use_bf16:
                        # SWDGE casts fp32 -> bf16 inline; HBM read traffic
                        # is unchanged, SBUF tile halves, and the DVE
                        # multiply gets the 2x bf16 perf mode.
                        nc.gpsimd.dma_start(out=xt, in_=src)
                    else:
                        # Write through an f32r-typed AP: the fp32r matmuls
                        # below require their producer to emit fp32r (PE
                        # truncates to FP22 on read; bits are plain fp32).
                        nc.sync.dma_start(out=mm(xt), in_=mm(src))
                    # scores for all super_ chunks of this superchunk,
                    # one column each; exp'd in a single ACT op.
                    scores = small.tile([P, super_], f32, tag="scores")
                    for j in range(super_):
                        y = ypool.tile([P, d], xd, tag="y")
                        nc.vector.tensor_mul(y, xt[:, j, :], w_sb)
                        nc.scalar.activation(
                            y,
                            y,
                            mybir.ActivationFunctionType.Copy,
                            accum_out=scores[:, j : j + 1],
                        )
                    col0 = b * cpb + sc * super_
                    nc.vector.tensor_add(
                        scores, scores, bias_sb[:, col0 : col0 + super_]
                    )
                    e = small.tile([P, super_], xd, tag="e")
                    er = mm(e)
                    nc.scalar.activation(
                        er, scores, mybir.ActivationFunctionType.Exp
                    )
                    for j in range(super_):
                        c = sc * super_ + j
                        first = c == 0
                        last = c == cpb - 1
                        ej = er[:, j : j + 1]
                        nc.tensor.matmul(
                            acc0,
                            ej,
                            mm(xt[:, j, :n_half]),
                            start=first,
                            stop=last,
                        )
                        nc.tensor.matmul(
                            acc1,
                            ej,
                            mm(xt[:, j, n_half:]),
                            start=first,
                            stop=last,
                        )
                        nc.tensor.matmul(
                            lps,
                            ej,
                            mm(ones_sb),
                            start=first,
                            stop=last,
                        )
                linv = small.tile([1, 1], f32, tag="linv")
                nc.vector.reciprocal(linv, lps[:, 0:1])
                ob = outp.tile([1, d], f32, tag="ob")
                nc.vector.tensor_scalar_mul(ob[:, :n_half], acc0, linv)
                nc.vector.tensor_scalar_mul(ob[:, n_half:], acc1, linv)
                nc.sync.dma_start(out=out_d[b : b + 1, :], in_=ob)
    nc.compile()
    return nc


def make_in_maps(x, padding_mask, w, b_pc=B_PC, s=S, d=D, n_cores=N_CORES,
                 use_bf16=False):
    """Shard inputs and build per-core host-side tensors."""
    x = np.asarray(x, dtype=np.float32)
    padding_mask = np.asarray(padding_mask)
    w = np.asarray(w, dtype=np.float32)
    cpb = s // P
    bias = np.where(padding_mask != 0, np.float32(0.0), np.float32(NEG_BIAS))
    bias = bias.astype(np.float32)
    w_rep = np.ascontiguousarray(np.broadcast_to(w.reshape(1, d), (P, d)))
    if use_bf16:
        import ml_dtypes
        w_rep = w_rep.astype(ml_dtypes.bfloat16)
    in_maps = []
    for core in range(n_cores):
        xc = np.ascontiguousarray(x[core * b_pc : (core + 1) * b_pc])
        bc = bias[core * b_pc : (core + 1) * b_pc]  # [b_pc, s]
        # bias_sb[p, b*cpb + c] = bias for row s = c*128 + p of batch b
        bc = np.ascontiguousarray(
            bc.reshape(b_pc, cpb, P).transpose(2, 0, 1).reshape(P, b_pc * cpb)
        )
        ones = np.ones((P, 2), dtype=np.float32)
        if use_bf16:
            import ml_dtypes
            ones = ones.astype(ml_dtypes.bfloat16)
        in_maps.append({"x": xc, "w_rep": w_rep, "bias": bc, "ones": ones})
    return in_maps


_NC_CACHE = {}


def _get_nc():
    if "nc" not in _NC_CACHE:
        _NC_CACHE["nc"] = build_bass()
    return _NC_CACHE["nc"]


def kernel(x, padding_mask, w):
    from concourse.bass_utils import run_bass_kernel_spmd

    nc = _get_nc()
    in_maps = make_in_maps(x, padding_mask, w)
    res = run_bass_kernel_spmd(nc, in_maps, list(range(N_CORES)))
    outs = [res.results[c]["out"] for c in range(N_CORES)]
    return np.concatenate(outs, axis=0).astype(np.float32)
